# revision 7
# baseline (speedup 1.0000x reference)
"""Multi-head attention (B=16, N=577, C=768, H=12) on 8 TRN2 NeuronCores.

Strategy: pure data parallelism over batch (2 images per core, no
collectives). Per core, everything is computed "channels-on-partitions"
(transposed) so that no on-device transposes are ever needed:

  qkT[outc, tok]  = qkv_wT-tiles.T @ xT          (q scaled 1/8 + bias on evict)
  V[tok, outc]    = xT-tiles.T @ qkv_wT          (natural layout, + bias)
  S^T[nk, nq]     = K^T-tiles.T @ Q^T            (K=64 contraction)
  E^T             = exp(S^T + relbT)             (no max subtraction; logits are
                                                  bounded ~|7| for this problem)
  O'^T[65, nq]    = [V_h | 1]-tiles.T @ E^T      (row 64 = softmax denominator)
  O^T             = O'^T[0:64] * bcast(1/O'^T[64])
  out^T[co, tok]  = projT-tiles.T @ O^T + proj_b

Host side pre-transposes all inputs (and converts to bf16) and transposes
the output back. PSUM accumulation is f32 throughout.
"""
import numpy as np
import ml_dtypes

B, N, C, H, HD = 16, 577, 768, 12, 64
NCORES = 8
BPC = B // NCORES          # batches per core: 2
NT = BPC * N               # tokens per core: 1154
P = 128

# token-free-dim chunks over NT (matmul free dim <= 512 for f32 psum)
TFREE = [(0, 512), (512, 512), (1024, 130)]
# nk (key token) tiles over N
NKT = [(0, 128), (128, 128), (256, 128), (384, 128), (512, 65)]
# nq (query token) chunks over N
NQF = [(0, 512), (512, 65)]

_CACHE = {}


def _build():
    import concourse.tile as tile
    from concourse import bacc, mybir

    bf16 = mybir.dt.bfloat16
    f32 = mybir.dt.float32
    Alu = mybir.AluOpType
    Act = mybir.ActivationFunctionType

    nc = bacc.Bacc(
        "TRN2",
        target_bir_lowering=False,
        debug=False,
        enable_asserts=False,
        num_devices=NCORES,
    )
    xT = nc.dram_tensor("xT", [C, NT], bf16, kind="ExternalInput").ap()
    wqkvT = nc.dram_tensor("wqkvT", [C, 3 * C], bf16, kind="ExternalInput").ap()
    qbias = nc.dram_tensor("qbias", [P, 6], f32, kind="ExternalInput").ap()
    vbias = nc.dram_tensor("vbias", [1, C], f32, kind="ExternalInput").ap()
    relbT = nc.dram_tensor("relbT", [H, N, N], bf16, kind="ExternalInput").ap()
    projT = nc.dram_tensor("projT", [C, C], bf16, kind="ExternalInput").ap()
    pbias = nc.dram_tensor("pbias", [P, 6], f32, kind="ExternalInput").ap()
    out = nc.dram_tensor("out", [C, NT], f32, kind="ExternalOutput").ap()

    with tile.TileContext(nc) as tc:
        with (
            tc.tile_pool(name="persist", bufs=1) as pp,
            tc.tile_pool(name="relb", bufs=2) as relp,
            tc.tile_pool(name="st", bufs=2) as stp,
            tc.tile_pool(name="et", bufs=2) as etp,
            tc.tile_pool(name="dn", bufs=6) as dnp,
            tc.tile_pool(name="oev", bufs=3) as oevp,
            tc.tile_pool(name="psmm", bufs=2, space="PSUM") as ps_mm,
            tc.tile_pool(name="pss", bufs=3, space="PSUM") as ps_s,
            tc.tile_pool(name="pso", bufs=3, space="PSUM") as ps_o,
        ):
            # ---------------- Phase A: load weights / constants ----------
            xt = []
            wt = []
            pt = []
            for i in range(6):
                t = pp.tile([P, NT], bf16, tag=f"xt{i}", name=f"xt{i}")
                nc.sync.dma_start(t[:], xT[P * i : P * (i + 1), :])
                xt.append(t)
                w = pp.tile([P, 3 * C], bf16, tag=f"wt{i}", name=f"wt{i}")
                nc.sync.dma_start(w[:], wqkvT[P * i : P * (i + 1), :])
                wt.append(w)
                pw = pp.tile([P, C], bf16, tag=f"pt{i}", name=f"pt{i}")
                nc.sync.dma_start(pw[:], projT[P * i : P * (i + 1), :])
                pt.append(pw)
            qb = pp.tile([P, 6], f32, tag="qb", name="qb")
            nc.sync.dma_start(qb[:], qbias[:])
            pb = pp.tile([P, 6], f32, tag="pb", name="pb")
            nc.sync.dma_start(pb[:], pbias[:])
            vbr = pp.tile([1, C], f32, tag="vbr", name="vbr")
            nc.sync.dma_start(vbr[:], vbias[:])
            vb = pp.tile([P, C], f32, tag="vb", name="vb")
            nc.gpsimd.partition_broadcast(vb[:, :], vbr[0:1, :])

            # ---------------- Phase B: Q^T / K^T projection ---------------
            # qk[t] for t in 0..11: [128, NT] bf16, outc block t (q: 0-5, k: 6-11)
            qk = []
            for t in range(12):
                qk.append(pp.tile([P, NT], bf16, tag=f"qk{t}", name=f"qk{t}"))
            for t in range(12):
                for (f0, fsz) in TFREE:
                    ps = ps_mm.tile([P, fsz], f32, tag="mm", name="psmm")
                    for ki in range(6):
                        nc.tensor.matmul(
                            ps[:, :],
                            wt[ki][:, P * t : P * (t + 1)],
                            xt[ki][:, f0 : f0 + fsz],
                            start=(ki == 0),
                            stop=(ki == 5),
                        )
                    if t < 6:  # q: scale 1/8 + bias (pre-scaled on host)
                        nc.scalar.activation(
                            qk[t][:, f0 : f0 + fsz],
                            ps[:, :],
                            Act.Identity,
                            bias=qb[:, t : t + 1],
                            scale=0.125,
                        )
                    else:  # k: plain copy (k bias is zero)
                        nc.scalar.copy(qk[t][:, f0 : f0 + fsz], ps[:, :])

            # ---------------- Phase C: V projection (natural layout) ------
            # v[b][j]: [nksz, 780] bf16, 12 head-blocks of 65 (64 V cols + ones)
            v = [[None] * 5 for _ in range(BPC)]
            for b in range(BPC):
                for j, (nk0, nksz) in enumerate(NKT):
                    vt = pp.tile([P, 12 * 65], bf16, tag=f"v{b}_{j}", name=f"v{b}_{j}")
                    v[b][j] = vt
                    v3 = vt[:, :].rearrange("p (h w) -> p h w", w=65)
                    nc.gpsimd.memset(v3[:, :, 64:65], 1.0)
                    tok0 = b * N + nk0
                    for half in range(2):  # outc halves of 384 = 6 heads
                        f0 = 384 * half
                        ps = ps_mm.tile([nksz, 384], f32, tag="mm", name="psmm")
                        for ki in range(6):
                            nc.tensor.matmul(
                                ps[:, :],
                                xt[ki][:, tok0 : tok0 + nksz],
                                wt[ki][:, 2 * C + f0 : 2 * C + f0 + 384],
                                start=(ki == 0),
                                stop=(ki == 5),
                            )
                        ps3 = ps[:, :].rearrange("p (h w) -> p h w", w=64)
                        vb3 = vb[0:nksz, f0 : f0 + 384].rearrange(
                            "p (h w) -> p h w", w=64
                        )
                        nc.vector.tensor_tensor(
                            v3[0:nksz, 6 * half : 6 * half + 6, 0:64],
                            ps3[:, :, :],
                            vb3[:, :, :],
                            op=Alu.add,
                        )

            # ---------------- Phase D: attention per (h, b) ----------------
            # o[t]: [128, NT] bf16 -- O^T assembled for the projection
            o = []
            for t in range(6):
                o.append(pp.tile([P, NT], bf16, tag=f"o{t}", name=f"o{t}"))
            for h in range(12):
                qt, qoff = h // 2, (h % 2) * 64
                relb = []
                for j, (nk0, nksz) in enumerate(NKT):
                    rt = relp.tile([P, N], bf16, tag=f"rb{j}", name=f"relb{j}")
                    nc.sync.dma_start(rt[0:nksz, :], relbT[h, nk0 : nk0 + nksz, :])
                    relb.append(rt)
                for b in range(BPC):
                    # S^T = K^T-tiles.T @ Q^T ; E^T = exp(S^T + relbT)
                    et = []
                    for j, (nk0, nksz) in enumerate(NKT):
                        e = etp.tile([P, N], bf16, tag=f"e{j}", name=f"et{j}")
                        et.append(e)
                        s = stp.tile([P, N], bf16, tag=f"s{j}", name=f"st{j}")
                        for (q0, qsz) in NQF:
                            ps = ps_s.tile([P, 512], f32, tag="s", name="pss")
                            nc.tensor.matmul(
                                ps[0:nksz, 0:qsz],
                                qk[6 + qt][qoff : qoff + 64, b * N + nk0 : b * N + nk0 + nksz],
                                qk[qt][qoff : qoff + 64, b * N + q0 : b * N + q0 + qsz],
                                start=True,
                                stop=True,
                            )
                            # exp-evict on ACT, then multiplicative bias
                            # (relb holds exp(bias^T)) on DVE in fast bf16 mode
                            nc.scalar.activation(
                                s[0:nksz, q0 : q0 + qsz],
                                ps[0:nksz, 0:qsz],
                                Act.Exp,
                            )
                            nc.vector.tensor_tensor(
                                e[0:nksz, q0 : q0 + qsz],
                                s[0:nksz, q0 : q0 + qsz],
                                relb[j][0:nksz, q0 : q0 + qsz],
                                op=Alu.mult,
                            )
                    # O'^T = [V_h | 1]-tiles.T @ E^T, then normalize
                    for (q0, qsz) in NQF:
                        ps = ps_o.tile([65, 512], f32, tag="o", name="pso")
                        for j, (nk0, nksz) in enumerate(NKT):
                            nc.tensor.matmul(
                                ps[0:65, 0:qsz],
                                v[b][j][0:nksz, 65 * h : 65 * h + 65],
                                et[j][0:nksz, q0 : q0 + qsz],
                                start=(j == 0),
                                stop=(j == 4),
                            )
                        dr = dnp.tile([1, 512], f32, tag="dr", name="dr")
                        nc.scalar.copy(dr[0:1, 0:qsz], ps[64:65, 0:qsz])
                        rr = dnp.tile([1, 512], f32, tag="rr", name="rr")
                        nc.vector.reciprocal_approx_fast(rr[0:1, 0:qsz], dr[0:1, 0:qsz])
                        rb = dnp.tile([64, 512], f32, tag="rbb", name="rbb")
                        nc.gpsimd.partition_broadcast(rb[0:64, 0:qsz], rr[0:1, 0:qsz])
                        nc.vector.tensor_tensor(
                            o[qt][qoff : qoff + 64, b * N + q0 : b * N + q0 + qsz],
                            ps[0:64, 0:qsz],
                            rb[0:64, 0:qsz],
                            op=Alu.mult,
                        )

            # ---------------- Phase E: output projection -------------------
            for t in range(6):
                for (f0, fsz) in TFREE:
                    ps = ps_mm.tile([P, fsz], f32, tag="mm", name="psmm")
                    for ki in range(6):
                        nc.tensor.matmul(
                            ps[:, :],
                            pt[ki][:, P * t : P * (t + 1)],
                            o[ki][:, f0 : f0 + fsz],
                            start=(ki == 0),
                            stop=(ki == 5),
                        )
                    ot = oevp.tile([P, 512], f32, tag="oev", name="oev")
                    nc.scalar.activation(
                        ot[:, 0:fsz], ps[:, :], Act.Identity, bias=pb[:, t : t + 1]
                    )
                    nc.sync.dma_start(out[P * t : P * (t + 1), f0 : f0 + fsz], ot[:, 0:fsz])

    nc.compile()
    return nc


def _get_nc():
    if "nc" not in _CACHE:
        _CACHE["nc"] = _build()
    return _CACHE["nc"]


def make_in_maps(x, rel_pos_bias, qkv_w, q_bias, v_bias, proj_w, proj_b):
    bf = ml_dtypes.bfloat16
    x = np.asarray(x, dtype=np.float32)
    rel_pos_bias = np.asarray(rel_pos_bias, dtype=np.float32)
    qkv_w = np.asarray(qkv_w, dtype=np.float32)
    q_bias = np.asarray(q_bias, dtype=np.float32)
    v_bias = np.asarray(v_bias, dtype=np.float32)
    proj_w = np.asarray(proj_w, dtype=np.float32)
    proj_b = np.asarray(proj_b, dtype=np.float32)

    wqkvT = np.ascontiguousarray(qkv_w.T).astype(bf)                    # [768, 2304]
    qbias = np.ascontiguousarray((q_bias * 0.125).reshape(6, P).T)      # [128, 6]
    vbias = np.ascontiguousarray(v_bias[None, :])                       # [1, 768]
    # exp of the transposed rel-pos bias: applied multiplicatively after exp(S)
    relbT = np.ascontiguousarray(
        np.exp(rel_pos_bias[0].transpose(0, 2, 1))
    ).astype(bf)
    projT = np.ascontiguousarray(proj_w.T).astype(bf)                   # [768, 768]
    pbias = np.ascontiguousarray(proj_b.reshape(6, P).T)                # [128, 6]

    in_maps = []
    for c in range(NCORES):
        xT = np.ascontiguousarray(
            x[BPC * c : BPC * (c + 1)].reshape(NT, C).T
        ).astype(bf)                                                    # [768, 1154]
        in_maps.append(
            dict(
                xT=xT,
                wqkvT=wqkvT,
                qbias=qbias,
                vbias=vbias,
                relbT=relbT,
                projT=projT,
                pbias=pbias,
            )
        )
    return in_maps


def kernel(x, rel_pos_bias, qkv_w, q_bias, v_bias, proj_w, proj_b):
    from concourse import bass_utils

    in_maps = make_in_maps(x, rel_pos_bias, qkv_w, q_bias, v_bias, proj_w, proj_b)
    nc = _get_nc()
    res = bass_utils.run_bass_kernel_spmd(nc, in_maps, core_ids=list(range(NCORES)))
    outs = []
    for c in range(NCORES):
        oT = res.results[c]["out"]                                      # [768, 1154]
        outs.append(np.ascontiguousarray(oT.T).reshape(BPC, N, C))
    return np.concatenate(outs, axis=0)


# revision 9
# speedup vs baseline: 1.0205x; 1.0205x over previous
"""Multi-head attention (B=16, N=577, C=768, H=12) on 8 TRN2 NeuronCores.

Strategy: pure data parallelism over batch (2 images per core, no
collectives). Per core, everything is computed "channels-on-partitions"
(transposed) so that no on-device transposes are ever needed:

  qkT[outc, tok]  = qkv_wT-tiles.T @ xT          (q scaled 1/8 + bias on evict)
  V[tok, outc]    = xT-tiles.T @ qkv_wT          (natural layout, + bias)
  S^T[nk, nq]     = K^T-tiles.T @ Q^T            (K=64 contraction)
  E^T             = exp(S^T + relbT)             (no max subtraction; logits are
                                                  bounded ~|7| for this problem)
  O'^T[65, nq]    = [V_h | 1]-tiles.T @ E^T      (row 64 = softmax denominator)
  O^T             = O'^T[0:64] * bcast(1/O'^T[64])
  out^T[co, tok]  = projT-tiles.T @ O^T + proj_b

Host side pre-transposes all inputs (and converts to bf16) and transposes
the output back. PSUM accumulation is f32 throughout.
"""
import numpy as np
import ml_dtypes

B, N, C, H, HD = 16, 577, 768, 12, 64
NCORES = 8
BPC = B // NCORES          # batches per core: 2
NT = BPC * N               # tokens per core: 1154
P = 128

# token-free-dim chunks over NT (matmul free dim <= 512 for f32 psum)
TFREE = [(0, 512), (512, 512), (1024, 130)]
# nk (key token) tiles over N
NKT = [(0, 128), (128, 128), (256, 128), (384, 128), (512, 65)]
# nq (query token) chunks over N
NQF = [(0, 512), (512, 65)]

_CACHE = {}


def _build():
    import concourse.tile as tile
    from concourse import bacc, mybir

    bf16 = mybir.dt.bfloat16
    f32 = mybir.dt.float32
    Alu = mybir.AluOpType
    Act = mybir.ActivationFunctionType

    nc = bacc.Bacc(
        "TRN2",
        target_bir_lowering=False,
        debug=False,
        enable_asserts=False,
        num_devices=NCORES,
    )
    xT = nc.dram_tensor("xT", [C, NT], bf16, kind="ExternalInput").ap()
    wqkvT = nc.dram_tensor("wqkvT", [C, 3 * C], bf16, kind="ExternalInput").ap()
    qbias = nc.dram_tensor("qbias", [P, 6], f32, kind="ExternalInput").ap()
    vbias = nc.dram_tensor("vbias", [1, C], f32, kind="ExternalInput").ap()
    relbT = nc.dram_tensor("relbT", [H, N, N], bf16, kind="ExternalInput").ap()
    projT = nc.dram_tensor("projT", [C, C], bf16, kind="ExternalInput").ap()
    pbias = nc.dram_tensor("pbias", [P, 6], f32, kind="ExternalInput").ap()
    out = nc.dram_tensor("out", [C, NT], f32, kind="ExternalOutput").ap()

    with tile.TileContext(nc) as tc:
        with (
            tc.tile_pool(name="persist", bufs=1) as pp,
            tc.tile_pool(name="relb", bufs=3) as relp,
            tc.tile_pool(name="st", bufs=3) as stp,
            tc.tile_pool(name="et", bufs=3) as etp,
            tc.tile_pool(name="dn", bufs=6) as dnp,
            tc.tile_pool(name="oev", bufs=3) as oevp,
            tc.tile_pool(name="psmm", bufs=2, space="PSUM") as ps_mm,
            tc.tile_pool(name="pss", bufs=3, space="PSUM") as ps_s,
            tc.tile_pool(name="pso", bufs=3, space="PSUM") as ps_o,
        ):
            # ---------------- Phase A: load weights / constants ----------
            xt = []
            wt = []
            pt = []
            for i in range(6):
                t = pp.tile([P, NT], bf16, tag=f"xt{i}", name=f"xt{i}")
                nc.sync.dma_start(t[:], xT[P * i : P * (i + 1), :])
                xt.append(t)
                w = pp.tile([P, 3 * C], bf16, tag=f"wt{i}", name=f"wt{i}")
                nc.sync.dma_start(w[:], wqkvT[P * i : P * (i + 1), :])
                wt.append(w)
                pw = pp.tile([P, C], bf16, tag=f"pt{i}", name=f"pt{i}")
                nc.sync.dma_start(pw[:], projT[P * i : P * (i + 1), :])
                pt.append(pw)
            qb = pp.tile([P, 6], f32, tag="qb", name="qb")
            nc.sync.dma_start(qb[:], qbias[:])
            pb = pp.tile([P, 6], f32, tag="pb", name="pb")
            nc.sync.dma_start(pb[:], pbias[:])
            vbr = pp.tile([1, C], f32, tag="vbr", name="vbr")
            nc.sync.dma_start(vbr[:], vbias[:])
            vb = pp.tile([P, C], f32, tag="vb", name="vb")
            nc.gpsimd.partition_broadcast(vb[:, :], vbr[0:1, :])

            # ---------------- Phase B: Q^T / K^T projection ---------------
            # qk[t] for t in 0..11: [128, NT] bf16, outc block t (q: 0-5, k: 6-11)
            qk = []
            for t in range(12):
                qk.append(pp.tile([P, NT], bf16, tag=f"qk{t}", name=f"qk{t}"))
            for t in range(12):
                for (f0, fsz) in TFREE:
                    ps = ps_mm.tile([P, fsz], f32, tag="mm", name="psmm")
                    for ki in range(6):
                        nc.tensor.matmul(
                            ps[:, :],
                            wt[ki][:, P * t : P * (t + 1)],
                            xt[ki][:, f0 : f0 + fsz],
                            start=(ki == 0),
                            stop=(ki == 5),
                        )
                    if t < 6:  # q: scale 1/8 + bias (pre-scaled on host)
                        nc.scalar.activation(
                            qk[t][:, f0 : f0 + fsz],
                            ps[:, :],
                            Act.Identity,
                            bias=qb[:, t : t + 1],
                            scale=0.125,
                        )
                    else:  # k: plain copy (k bias is zero)
                        nc.scalar.copy(qk[t][:, f0 : f0 + fsz], ps[:, :])

            # ---------------- Phase C: V projection (natural layout) ------
            # v[b][j]: [nksz, 780] bf16, 12 head-blocks of 65 (64 V cols + ones)
            v = [[None] * 5 for _ in range(BPC)]
            for b in range(BPC):
                for j, (nk0, nksz) in enumerate(NKT):
                    vt = pp.tile([P, 12 * 65], bf16, tag=f"v{b}_{j}", name=f"v{b}_{j}")
                    v[b][j] = vt
                    v3 = vt[:, :].rearrange("p (h w) -> p h w", w=65)
                    nc.gpsimd.memset(v3[:, :, 64:65], 1.0)
                    tok0 = b * N + nk0
                    for half in range(2):  # outc halves of 384 = 6 heads
                        f0 = 384 * half
                        ps = ps_mm.tile([nksz, 384], f32, tag="mm", name="psmm")
                        for ki in range(6):
                            nc.tensor.matmul(
                                ps[:, :],
                                xt[ki][:, tok0 : tok0 + nksz],
                                wt[ki][:, 2 * C + f0 : 2 * C + f0 + 384],
                                start=(ki == 0),
                                stop=(ki == 5),
                            )
                        ps3 = ps[:, :].rearrange("p (h w) -> p h w", w=64)
                        vb3 = vb[0:nksz, f0 : f0 + 384].rearrange(
                            "p (h w) -> p h w", w=64
                        )
                        nc.vector.tensor_tensor(
                            v3[0:nksz, 6 * half : 6 * half + 6, 0:64],
                            ps3[:, :, :],
                            vb3[:, :, :],
                            op=Alu.add,
                        )

            # ---------------- Phase D: attention per (h, b) ----------------
            # o[t]: [128, NT] bf16 -- O^T assembled for the projection
            o = []
            for t in range(6):
                o.append(pp.tile([P, NT], bf16, tag=f"o{t}", name=f"o{t}"))
            for h in range(12):
                qt, qoff = h // 2, (h % 2) * 64
                relb = []
                for j, (nk0, nksz) in enumerate(NKT):
                    rt = relp.tile([P, N], bf16, tag=f"rb{j}", name=f"relb{j}")
                    nc.sync.dma_start(rt[0:nksz, :], relbT[h, nk0 : nk0 + nksz, :])
                    relb.append(rt)
                for b in range(BPC):
                    # S^T = K^T-tiles.T @ Q^T ; E^T = exp(S^T + relbT)
                    et = []
                    for j, (nk0, nksz) in enumerate(NKT):
                        e = etp.tile([P, N], bf16, tag=f"e{j}", name=f"et{j}")
                        et.append(e)
                        s = stp.tile([P, N], bf16, tag=f"s{j}", name=f"st{j}")
                        for ci, (q0, qsz) in enumerate(NQF):
                            # alternate between two psum pools so up to 5
                            # S tiles are in flight (phase B/C's pool is idle
                            # during attention)
                            psp_pick = ps_s if (2 * j + ci) % 2 == 0 else ps_mm
                            ps = psp_pick.tile(
                                [P, 512], f32,
                                tag="s" if psp_pick is ps_s else "mm",
                                name="pss",
                            )
                            nc.tensor.matmul(
                                ps[0:nksz, 0:qsz],
                                qk[6 + qt][qoff : qoff + 64, b * N + nk0 : b * N + nk0 + nksz],
                                qk[qt][qoff : qoff + 64, b * N + q0 : b * N + q0 + qsz],
                                start=True,
                                stop=True,
                            )
                            # exp-evict on ACT, then multiplicative bias
                            # (relb holds exp(bias^T)) on DVE in fast bf16 mode
                            nc.scalar.activation(
                                s[0:nksz, q0 : q0 + qsz],
                                ps[0:nksz, 0:qsz],
                                Act.Exp,
                            )
                            nc.vector.tensor_tensor(
                                e[0:nksz, q0 : q0 + qsz],
                                s[0:nksz, q0 : q0 + qsz],
                                relb[j][0:nksz, q0 : q0 + qsz],
                                op=Alu.mult,
                            )
                    # O'^T = [V_h | 1]-tiles.T @ E^T, then normalize
                    for (q0, qsz) in NQF:
                        ps = ps_o.tile([65, 512], f32, tag="o", name="pso")
                        for j, (nk0, nksz) in enumerate(NKT):
                            nc.tensor.matmul(
                                ps[0:65, 0:qsz],
                                v[b][j][0:nksz, 65 * h : 65 * h + 65],
                                et[j][0:nksz, q0 : q0 + qsz],
                                start=(j == 0),
                                stop=(j == 4),
                            )
                        dr = dnp.tile([1, 512], f32, tag="dr", name="dr")
                        nc.scalar.copy(dr[0:1, 0:qsz], ps[64:65, 0:qsz])
                        rr = dnp.tile([1, 512], f32, tag="rr", name="rr")
                        nc.vector.reciprocal_approx_fast(rr[0:1, 0:qsz], dr[0:1, 0:qsz])
                        rb = dnp.tile([64, 512], f32, tag="rbb", name="rbb")
                        nc.gpsimd.partition_broadcast(rb[0:64, 0:qsz], rr[0:1, 0:qsz])
                        nc.vector.tensor_tensor(
                            o[qt][qoff : qoff + 64, b * N + q0 : b * N + q0 + qsz],
                            ps[0:64, 0:qsz],
                            rb[0:64, 0:qsz],
                            op=Alu.mult,
                        )

            # ---------------- Phase E: output projection -------------------
            for t in range(6):
                for (f0, fsz) in TFREE:
                    ps = ps_mm.tile([P, fsz], f32, tag="mm", name="psmm")
                    for ki in range(6):
                        nc.tensor.matmul(
                            ps[:, :],
                            pt[ki][:, P * t : P * (t + 1)],
                            o[ki][:, f0 : f0 + fsz],
                            start=(ki == 0),
                            stop=(ki == 5),
                        )
                    ot = oevp.tile([P, 512], f32, tag="oev", name="oev")
                    nc.scalar.activation(
                        ot[:, 0:fsz], ps[:, :], Act.Identity, bias=pb[:, t : t + 1]
                    )
                    nc.sync.dma_start(out[P * t : P * (t + 1), f0 : f0 + fsz], ot[:, 0:fsz])

    nc.compile()
    return nc


def _get_nc():
    if "nc" not in _CACHE:
        _CACHE["nc"] = _build()
    return _CACHE["nc"]


def make_in_maps(x, rel_pos_bias, qkv_w, q_bias, v_bias, proj_w, proj_b):
    bf = ml_dtypes.bfloat16
    x = np.asarray(x, dtype=np.float32)
    rel_pos_bias = np.asarray(rel_pos_bias, dtype=np.float32)
    qkv_w = np.asarray(qkv_w, dtype=np.float32)
    q_bias = np.asarray(q_bias, dtype=np.float32)
    v_bias = np.asarray(v_bias, dtype=np.float32)
    proj_w = np.asarray(proj_w, dtype=np.float32)
    proj_b = np.asarray(proj_b, dtype=np.float32)

    wqkvT = np.ascontiguousarray(qkv_w.T).astype(bf)                    # [768, 2304]
    qbias = np.ascontiguousarray((q_bias * 0.125).reshape(6, P).T)      # [128, 6]
    vbias = np.ascontiguousarray(v_bias[None, :])                       # [1, 768]
    # exp of the transposed rel-pos bias: applied multiplicatively after exp(S)
    relbT = np.ascontiguousarray(
        np.exp(rel_pos_bias[0].transpose(0, 2, 1))
    ).astype(bf)
    projT = np.ascontiguousarray(proj_w.T).astype(bf)                   # [768, 768]
    pbias = np.ascontiguousarray(proj_b.reshape(6, P).T)                # [128, 6]

    in_maps = []
    for c in range(NCORES):
        xT = np.ascontiguousarray(
            x[BPC * c : BPC * (c + 1)].reshape(NT, C).T
        ).astype(bf)                                                    # [768, 1154]
        in_maps.append(
            dict(
                xT=xT,
                wqkvT=wqkvT,
                qbias=qbias,
                vbias=vbias,
                relbT=relbT,
                projT=projT,
                pbias=pbias,
            )
        )
    return in_maps


def kernel(x, rel_pos_bias, qkv_w, q_bias, v_bias, proj_w, proj_b):
    from concourse import bass_utils

    in_maps = make_in_maps(x, rel_pos_bias, qkv_w, q_bias, v_bias, proj_w, proj_b)
    nc = _get_nc()
    res = bass_utils.run_bass_kernel_spmd(nc, in_maps, core_ids=list(range(NCORES)))
    outs = []
    for c in range(NCORES):
        oT = res.results[c]["out"]                                      # [768, 1154]
        outs.append(np.ascontiguousarray(oT.T).reshape(BPC, N, C))
    return np.concatenate(outs, axis=0)


# revision 12
# speedup vs baseline: 1.0468x; 1.0258x over previous
"""Multi-head attention (B=16, N=577, C=768, H=12) on 8 TRN2 NeuronCores.

Strategy: pure data parallelism over batch (2 images per core, no
collectives). Per core, everything is computed "channels-on-partitions"
(transposed) so that no on-device transposes are ever needed:

  qkT[outc, tok]  = qkv_wT-tiles.T @ xT          (q scaled 1/8 + bias on evict)
  V[tok, outc]    = xT-tiles.T @ qkv_wT          (natural layout, + bias)
  S^T[nk, nq]     = K^T-tiles.T @ Q^T            (K=64 contraction)
  E^T             = exp(S^T + relbT)             (no max subtraction; logits are
                                                  bounded ~|7| for this problem)
  O'^T[65, nq]    = [V_h | 1]-tiles.T @ E^T      (row 64 = softmax denominator)
  O^T             = O'^T[0:64] * bcast(1/O'^T[64])
  out^T[co, tok]  = projT-tiles.T @ O^T + proj_b

Host side pre-transposes all inputs (and converts to bf16) and transposes
the output back. PSUM accumulation is f32 throughout.
"""
import numpy as np
import ml_dtypes

B, N, C, H, HD = 16, 577, 768, 12, 64
NCORES = 8
BPC = B // NCORES          # batches per core: 2
NT = BPC * N               # tokens per core: 1154
P = 128

# token-free-dim chunks over NT (matmul free dim <= 512 for f32 psum)
TFREE = [(0, 512), (512, 512), (1024, 130)]
# nk (key token) tiles over N
NKT = [(0, 128), (128, 128), (256, 128), (384, 128), (512, 65)]
# nq (query token) chunks over N
NQF = [(0, 512), (512, 65)]

_CACHE = {}


def _build():
    import concourse.tile as tile
    from concourse import bacc, mybir

    bf16 = mybir.dt.bfloat16
    f32 = mybir.dt.float32
    Alu = mybir.AluOpType
    Act = mybir.ActivationFunctionType

    nc = bacc.Bacc(
        "TRN2",
        target_bir_lowering=False,
        debug=False,
        enable_asserts=False,
        num_devices=NCORES,
    )
    xT = nc.dram_tensor("xT", [C, NT], bf16, kind="ExternalInput").ap()
    wqkvT = nc.dram_tensor("wqkvT", [C, 3 * C], bf16, kind="ExternalInput").ap()
    qbias = nc.dram_tensor("qbias", [P, 6], f32, kind="ExternalInput").ap()
    vbias = nc.dram_tensor("vbias", [1, C], f32, kind="ExternalInput").ap()
    relbT = nc.dram_tensor("relbT", [H, N, N], bf16, kind="ExternalInput").ap()
    projT = nc.dram_tensor("projT", [C, C], bf16, kind="ExternalInput").ap()
    pbias = nc.dram_tensor("pbias", [P, 6], f32, kind="ExternalInput").ap()
    out = nc.dram_tensor("out", [C, NT], f32, kind="ExternalOutput").ap()

    with tile.TileContext(nc) as tc:
        with (
            tc.tile_pool(name="persist", bufs=1) as pp,
            tc.tile_pool(name="relb", bufs=3) as relp,
            tc.tile_pool(name="st", bufs=2) as stp,
            tc.tile_pool(name="et", bufs=2) as etp,
            tc.tile_pool(name="dn", bufs=3) as dnp,
            tc.tile_pool(name="oev", bufs=3) as oevp,
            tc.tile_pool(name="psmm", bufs=2, space="PSUM") as ps_mm,
            tc.tile_pool(name="pss", bufs=3, space="PSUM") as ps_s,
            tc.tile_pool(name="pso", bufs=3, space="PSUM") as ps_o,
        ):
            # ---------------- Phase A: load weights / constants ----------
            xt = []
            wt = []
            pt = []
            for i in range(6):
                t = pp.tile([P, NT], bf16, tag=f"xt{i}", name=f"xt{i}")
                nc.sync.dma_start(t[:], xT[P * i : P * (i + 1), :])
                xt.append(t)
                w = pp.tile([P, 3 * C], bf16, tag=f"wt{i}", name=f"wt{i}")
                nc.sync.dma_start(w[:], wqkvT[P * i : P * (i + 1), :])
                wt.append(w)
                pw = pp.tile([P, C], bf16, tag=f"pt{i}", name=f"pt{i}")
                nc.sync.dma_start(pw[:], projT[P * i : P * (i + 1), :])
                pt.append(pw)
            qb = pp.tile([P, 6], f32, tag="qb", name="qb")
            nc.sync.dma_start(qb[:], qbias[:])
            pb = pp.tile([P, 6], f32, tag="pb", name="pb")
            nc.sync.dma_start(pb[:], pbias[:])
            vbr = pp.tile([1, C], f32, tag="vbr", name="vbr")
            nc.sync.dma_start(vbr[:], vbias[:])
            vb = pp.tile([P, C], f32, tag="vb", name="vb")
            nc.gpsimd.partition_broadcast(vb[:, :], vbr[0:1, :])

            # ---------------- Phase B: Q^T / K^T projection ---------------
            # qk[t] for t in 0..11: [128, NT] bf16, outc block t (q: 0-5, k: 6-11)
            qk = []
            for t in range(12):
                qk.append(pp.tile([P, NT], bf16, tag=f"qk{t}", name=f"qk{t}"))
            for t in range(12):
                for (f0, fsz) in TFREE:
                    ps = ps_mm.tile([P, fsz], f32, tag="mm", name="psmm")
                    for ki in range(6):
                        nc.tensor.matmul(
                            ps[:, :],
                            wt[ki][:, P * t : P * (t + 1)],
                            xt[ki][:, f0 : f0 + fsz],
                            start=(ki == 0),
                            stop=(ki == 5),
                        )
                    if t < 6:  # q: scale 1/8 + bias (pre-scaled on host)
                        nc.scalar.activation(
                            qk[t][:, f0 : f0 + fsz],
                            ps[:, :],
                            Act.Identity,
                            bias=qb[:, t : t + 1],
                            scale=0.125,
                        )
                    else:  # k: plain copy (k bias is zero)
                        nc.scalar.copy(qk[t][:, f0 : f0 + fsz], ps[:, :])

            # ---------------- Phase C: V projection (natural layout) ------
            # v[b][j]: [nksz, 780] bf16, 12 head-blocks of 65 (64 V cols + ones)
            v = [[None] * 5 for _ in range(BPC)]
            for b in range(BPC):
                for j, (nk0, nksz) in enumerate(NKT):
                    vt = pp.tile([P, 12 * 65], bf16, tag=f"v{b}_{j}", name=f"v{b}_{j}")
                    v[b][j] = vt
                    v3 = vt[:, :].rearrange("p (h w) -> p h w", w=65)
                    nc.gpsimd.memset(v3[:, :, 64:65], 1.0)
                    tok0 = b * N + nk0
                    for half in range(2):  # outc halves of 384 = 6 heads
                        f0 = 384 * half
                        ps = ps_mm.tile([nksz, 384], f32, tag="mm", name="psmm")
                        for ki in range(6):
                            nc.tensor.matmul(
                                ps[:, :],
                                xt[ki][:, tok0 : tok0 + nksz],
                                wt[ki][:, 2 * C + f0 : 2 * C + f0 + 384],
                                start=(ki == 0),
                                stop=(ki == 5),
                            )
                        ps3 = ps[:, :].rearrange("p (h w) -> p h w", w=64)
                        vb3 = vb[0:nksz, f0 : f0 + 384].rearrange(
                            "p (h w) -> p h w", w=64
                        )
                        nc.vector.tensor_tensor(
                            v3[0:nksz, 6 * half : 6 * half + 6, 0:64],
                            ps3[:, :, :],
                            vb3[:, :, :],
                            op=Alu.add,
                        )

            # ---------------- Phase D: attention per (h, b) ----------------
            # o[t]: [128, NT] bf16 -- O^T assembled for the projection
            o = []
            for t in range(6):
                o.append(pp.tile([P, NT], bf16, tag=f"o{t}", name=f"o{t}"))
            for h in range(12):
                qt, qoff = h // 2, (h % 2) * 64
                relb = []
                for j, (nk0, nksz) in enumerate(NKT):
                    rt = relp.tile([P, N], bf16, tag=f"rb{j}", name=f"relb{j}")
                    nc.sync.dma_start(rt[0:nksz, :], relbT[h, nk0 : nk0 + nksz, :])
                    relb.append(rt)
                # S^T for BOTH batches first, then O' for both: while O'(b0)
                # waits on the exp/mult chain, the PE streams S(b1) matmuls.
                et_b = []
                for b in range(BPC):
                    et = []
                    for j, (nk0, nksz) in enumerate(NKT):
                        e = etp.tile([P, N], bf16, tag=f"e{b}_{j}", name=f"et{b}_{j}")
                        et.append(e)
                        s = stp.tile([P, N], bf16, tag=f"s{b}_{j}", name=f"st{b}_{j}")
                        for ci, (q0, qsz) in enumerate(NQF):
                            # alternate between two psum pools so up to 5
                            # S tiles are in flight (phase B/C's pool is idle
                            # during attention)
                            psp_pick = ps_s if (2 * j + ci) % 2 == 0 else ps_mm
                            ps = psp_pick.tile(
                                [P, 512], f32,
                                tag="s" if psp_pick is ps_s else "mm",
                                name="pss",
                            )
                            nc.tensor.matmul(
                                ps[0:nksz, 0:qsz],
                                qk[6 + qt][qoff : qoff + 64, b * N + nk0 : b * N + nk0 + nksz],
                                qk[qt][qoff : qoff + 64, b * N + q0 : b * N + q0 + qsz],
                                start=True,
                                stop=True,
                            )
                            # exp-evict on ACT, then multiplicative bias
                            # (relb holds exp(bias^T)) on DVE in fast bf16 mode
                            nc.scalar.activation(
                                s[0:nksz, q0 : q0 + qsz],
                                ps[0:nksz, 0:qsz],
                                Act.Exp,
                            )
                            nc.vector.tensor_tensor(
                                e[0:nksz, q0 : q0 + qsz],
                                s[0:nksz, q0 : q0 + qsz],
                                relb[j][0:nksz, q0 : q0 + qsz],
                                op=Alu.mult,
                            )
                    et_b.append(et)
                for b in range(BPC):
                    et = et_b[b]
                    # O'^T = [V_h | 1]-tiles.T @ E^T, then normalize
                    for (q0, qsz) in NQF:
                        ps = ps_o.tile([65, 512], f32, tag="o", name="pso")
                        for j, (nk0, nksz) in enumerate(NKT):
                            nc.tensor.matmul(
                                ps[0:65, 0:qsz],
                                v[b][j][0:nksz, 65 * h : 65 * h + 65],
                                et[j][0:nksz, q0 : q0 + qsz],
                                start=(j == 0),
                                stop=(j == 4),
                            )
                        dr = dnp.tile([1, 512], f32, tag="dr", name="dr")
                        nc.scalar.copy(dr[0:1, 0:qsz], ps[64:65, 0:qsz])
                        rr = dnp.tile([1, 512], f32, tag="rr", name="rr")
                        nc.vector.reciprocal_approx_fast(rr[0:1, 0:qsz], dr[0:1, 0:qsz])
                        rb = dnp.tile([64, 512], f32, tag="rbb", name="rbb")
                        nc.gpsimd.partition_broadcast(rb[0:64, 0:qsz], rr[0:1, 0:qsz])
                        nc.vector.tensor_tensor(
                            o[qt][qoff : qoff + 64, b * N + q0 : b * N + q0 + qsz],
                            ps[0:64, 0:qsz],
                            rb[0:64, 0:qsz],
                            op=Alu.mult,
                        )

            # ---------------- Phase E: output projection -------------------
            for t in range(6):
                for (f0, fsz) in TFREE:
                    ps = ps_mm.tile([P, fsz], f32, tag="mm", name="psmm")
                    for ki in range(6):
                        nc.tensor.matmul(
                            ps[:, :],
                            pt[ki][:, P * t : P * (t + 1)],
                            o[ki][:, f0 : f0 + fsz],
                            start=(ki == 0),
                            stop=(ki == 5),
                        )
                    ot = oevp.tile([P, 512], f32, tag="oev", name="oev")
                    nc.scalar.activation(
                        ot[:, 0:fsz], ps[:, :], Act.Identity, bias=pb[:, t : t + 1]
                    )
                    nc.sync.dma_start(out[P * t : P * (t + 1), f0 : f0 + fsz], ot[:, 0:fsz])

    nc.compile()
    return nc


def _get_nc():
    if "nc" not in _CACHE:
        _CACHE["nc"] = _build()
    return _CACHE["nc"]


def make_in_maps(x, rel_pos_bias, qkv_w, q_bias, v_bias, proj_w, proj_b):
    bf = ml_dtypes.bfloat16
    x = np.asarray(x, dtype=np.float32)
    rel_pos_bias = np.asarray(rel_pos_bias, dtype=np.float32)
    qkv_w = np.asarray(qkv_w, dtype=np.float32)
    q_bias = np.asarray(q_bias, dtype=np.float32)
    v_bias = np.asarray(v_bias, dtype=np.float32)
    proj_w = np.asarray(proj_w, dtype=np.float32)
    proj_b = np.asarray(proj_b, dtype=np.float32)

    wqkvT = np.ascontiguousarray(qkv_w.T).astype(bf)                    # [768, 2304]
    qbias = np.ascontiguousarray((q_bias * 0.125).reshape(6, P).T)      # [128, 6]
    vbias = np.ascontiguousarray(v_bias[None, :])                       # [1, 768]
    # exp of the transposed rel-pos bias: applied multiplicatively after exp(S)
    relbT = np.ascontiguousarray(
        np.exp(rel_pos_bias[0].transpose(0, 2, 1))
    ).astype(bf)
    projT = np.ascontiguousarray(proj_w.T).astype(bf)                   # [768, 768]
    pbias = np.ascontiguousarray(proj_b.reshape(6, P).T)                # [128, 6]

    in_maps = []
    for c in range(NCORES):
        xT = np.ascontiguousarray(
            x[BPC * c : BPC * (c + 1)].reshape(NT, C).T
        ).astype(bf)                                                    # [768, 1154]
        in_maps.append(
            dict(
                xT=xT,
                wqkvT=wqkvT,
                qbias=qbias,
                vbias=vbias,
                relbT=relbT,
                projT=projT,
                pbias=pbias,
            )
        )
    return in_maps


def kernel(x, rel_pos_bias, qkv_w, q_bias, v_bias, proj_w, proj_b):
    from concourse import bass_utils

    in_maps = make_in_maps(x, rel_pos_bias, qkv_w, q_bias, v_bias, proj_w, proj_b)
    nc = _get_nc()
    res = bass_utils.run_bass_kernel_spmd(nc, in_maps, core_ids=list(range(NCORES)))
    outs = []
    for c in range(NCORES):
        oT = res.results[c]["out"]                                      # [768, 1154]
        outs.append(np.ascontiguousarray(oT.T).reshape(BPC, N, C))
    return np.concatenate(outs, axis=0)
